# revision 14
# baseline (speedup 1.0000x reference)
"""Trainium2 Bass kernel for batched per-sample expert matmul (MoE routing).

Computes y[n, i] = relu(b[idxs[n], i] + sum_o w[idxs[n], i, o] * x[n, o])
for x (8192, 256), idxs (8192,), w (64, 256, 256), b (64, 256).

Strategy (v2: expert-sharded, weight-stationary)
------------------------------------------------
Host side (numpy, cheap):
  * Group samples by expert; cut per-expert slots of <= 256 samples
    (one PSUM bank each); LPT-deal slots to the 8 cores balancing
    sample count.  Each expert's weight block is loaded ONCE per core
    (~1 MB/core vs 1.5 MB for slot-replicated loads), x/y ~0.5 MB each.
  * All matmul operands fp16 (absmax-rel err ~4e-4, gate is 2e-2).
  * Static SPMD program: slot widths W_s = max over cores of the
    rank-s slot's sample count (slots sorted desc), zero-padded.

Device side (one static Tile program on all 8 cores):
  Per slot s (expert e, W samples):  psum tile [128, 2W] (one bank)
      ps[i, n]      = sum_o wT[o, i]   * xT[o, n]      (4 matmuls:
        (o0,i0) start, (o1,i0), (o0,i1), (o1,i1) stop; weights are
        the stationary operand so PE cost scales with W, not 256)
      y[i', n]      = max(ps + b_col, 0)               (fused bias+relu,
        per-partition scalar; alternating DVE / ACT engines)
  Orientation note: output is yT (features on partitions); host
  transposes back.

  DMA plan: first slot group rides sync HWDGE (low first-byte latency);
  the bulk stream goes as few large packed x+w group DMAs on the gpsimd
  SWDGE queue (FIFO, one completion sem per group; issue cost ~0.65us
  each so groups hold >= 2 slots); bias consts ride scalar HWDGE.
  y outputs ride scalar HWDGE as their relus complete; the final small
  y group rides sync for the shortest last-receipt.

Host side: scatter yT columns back to sample order.
"""

import os

import numpy as np

import concourse.bacc as bacc
import concourse.bass as bass
import concourse.mybir as mybir
import concourse.tile as tile
from concourse.bass_utils import run_bass_kernel_spmd

N_CORES = 8
P = 128          # SBUF/PSUM partitions
F = 256          # feature dim (in == out == 256)
NEXP = 64
WMAX = 256       # samples per slot cap: psum tile [128, 2*W] fp32 <= one bank
MM_NP = np.float16
MM_DT = mybir.dt.float16
OUT_DT = mybir.dt.float16
F32 = mybir.dt.float32

# Set by the last kernel() call when KBENCH_TRACE=1 (used by test.py only).
LAST_EXEC_TIME_NS = None
LAST_TRACE = None


def _build_schedule(idxs: np.ndarray):
    """Sort samples by expert, cut <=WMAX-sample single-expert slots,
    LPT-deal slots to cores balancing total samples; sort each core's
    slots by count desc and compute static per-rank widths."""
    B = idxs.shape[0]
    order = np.argsort(idxs, kind="stable")
    sidx = idxs[order]
    slots = []  # (expert, global_start, count)
    i = 0
    while i < B:
        j = i
        while j < B and sidx[j] == sidx[i]:
            j += 1
        k = i
        while k < j:
            cnt = min(WMAX, j - k)
            slots.append((int(sidx[i]), k, cnt))
            k += cnt
        i = j
    # serpentine rank dealing: sort slots desc, rank r takes slots
    # [8r, 8r+8) (adjacent sizes -> minimal per-rank padding), serpentine
    # direction alternation balances per-core totals
    slots.sort(key=lambda t: -t[2])
    S = (len(slots) + N_CORES - 1) // N_CORES
    while len(slots) < S * N_CORES:
        slots.append((0, 0, 0))
    per_core = [[] for _ in range(N_CORES)]
    for r in range(S):
        rank = slots[r * N_CORES:(r + 1) * N_CORES]
        if r % 2 == 1:
            rank = rank[::-1]
        for c in range(N_CORES):
            per_core[c].append(rank[c])
    # static width per rank: max count across cores, multiple of 8, >= 8
    widths = []
    for s in range(S):
        w = max(pc[s][2] for pc in per_core)
        widths.append(max(8, (w + 7) // 8 * 8))
    return order, per_core, widths


def _in_groups(S):
    """Input stream slot groups: [1 (sync), 2s on gpsimd, single tail]."""
    out = [(0, 1)]
    lo = 1
    while lo < S - 1:
        hi = min(S - 1, lo + 2)
        out.append((lo, hi))
        lo = hi
    if S > 1:
        out.append((S - 1, S))
    return out


def _out_groups(S):
    """Output slot groups: pairs, except a single-slot tail group."""
    out = []
    lo = 0
    while lo < S - 1:
        hi = min(S - 1, lo + 2)
        out.append((lo, hi))
        lo = hi
    out.append((S - 1, S))
    return out


def _build_program(S, widths):
    nc = bacc.Bacc(
        "TRN2", target_bir_lowering=False, debug=False, num_devices=N_CORES
    )
    WSLOT = 4 * P  # w cols per slot (4 chunks of [128,128])
    cols = [WSLOT + 2 * w for w in widths]  # per-slot packed w+x cols
    igroups = _in_groups(S)
    ogroups = _out_groups(S)
    goff = []
    off = 0
    for lo, hi in igroups:
        goff.append(off)
        off += sum(cols[lo:hi])
    XWTOT = off
    yoff = []
    off = 0
    for s in range(S):
        yoff.append(off)
        off += 2 * widths[s]
    YTOT = off

    xw_d = nc.dram_tensor("xw", [P, XWTOT], MM_DT, kind="ExternalInput").ap()
    bc_d = nc.dram_tensor("bconst", [P, 2 * S], F32, kind="ExternalInput").ap()
    y_d = nc.dram_tensor("y", [P, YTOT], OUT_DT, kind="ExternalOutput").ap()

    with tile.TileContext(nc) as tc:
        with (
            tc.tile_pool(name="const", bufs=1) as const,
            tc.tile_pool(name="w", bufs=1) as wpool,
            tc.tile_pool(name="yout", bufs=1) as ypool,
            tc.tile_pool(name="ps", bufs=min(8, S), space="PSUM") as pspool,
        ):
            # bias columns on the scalar HWDGE ring (tiny, first in queue)
            bc = const.tile([P, 2 * S], F32, tag="bconst")
            with tc.high_priority():
                nc.scalar.dma_start(bc[:], bc_d[:])

            # input groups interleave across BOTH HWDGE rings (sync: even
            # groups, scalar: odd): the SDMA engines round-robin between the
            # rings at packet granularity, filling each ring's per-group
            # handoff dips; only the LAST group's completion gates the tail
            gts = {}
            for g, (lo, hi) in enumerate(igroups):
                gw = sum(cols[lo:hi])
                t = wpool.tile([P, gw], MM_DT, tag=f"xw{g}", name=f"xw{g}")
                gts[g] = t
                eng = nc.sync if g % 2 == 0 else nc.scalar
                if g == len(igroups) - 1 and hi - lo == 1:
                    # split last group into w-part and x-part for an earlier
                    # completion sem on the critical tail
                    wpart = 4 * P
                    eng.dma_start(
                        t[:, 0:wpart], xw_d[:, goff[g]:goff[g] + wpart]
                    )
                    eng.dma_start(
                        t[:, wpart:gw],
                        xw_d[:, goff[g] + wpart:goff[g] + gw],
                    )
                else:
                    eng.dma_start(t[:], xw_d[:, goff[g]:goff[g] + gw])

            # dummy activation (after the dma issues): hoists the one-time
            # ACT table load into the stream shadow so the tail-slot
            # activations run table-resident
            scratch = const.tile([1, 1], F32, tag="actwarm")
            nc.scalar.activation(
                scratch[:], bc[0:1, 0:1],
                mybir.ActivationFunctionType.Relu,
            )

            seg2g = {}
            for g, (lo, hi) in enumerate(igroups):
                for s in range(lo, hi):
                    seg2g[s] = g

            def wchunk(s, c):
                g = seg2g[s]
                lo, hi = igroups[g]
                base = sum(cols[lo:s]) + c * P
                return gts[g][:, base:base + P]

            def xchunk(s, c):
                g = seg2g[s]
                lo, hi = igroups[g]
                w = widths[s]
                base = sum(cols[lo:s]) + WSLOT + c * w
                return gts[g][:, base:base + w]

            seg2o = {}
            for g, (lo, hi) in enumerate(ogroups):
                for s in range(lo, hi):
                    seg2o[s] = g
            yts = {}

            for s in range(S):
                w = widths[s]
                ps = pspool.tile([P, 512], F32, name="ps")
                # 4 matmuls: (o0,i0) start, (o1,i0), (o0,i1), (o1,i1) stop
                nc.tensor.matmul(
                    ps[:, 0:w], wchunk(s, 0), xchunk(s, 0),
                    start=True, stop=False,
                )
                nc.tensor.matmul(
                    ps[:, 0:w], wchunk(s, 1), xchunk(s, 1),
                    start=False, stop=False,
                )
                nc.tensor.matmul(
                    ps[:, w:2 * w], wchunk(s, 2), xchunk(s, 0),
                    start=False, stop=False,
                )
                nc.tensor.matmul(
                    ps[:, w:2 * w], wchunk(s, 3), xchunk(s, 1),
                    start=False, stop=True,
                )

                og = seg2o[s]
                olo, ohi = ogroups[og]
                if og not in yts:
                    gw = sum(2 * widths[t_] for t_ in range(olo, ohi))
                    yts[og] = ypool.tile(
                        [P, gw], OUT_DT, tag=f"y{og}", name=f"y{og}"
                    )
                yt = yts[og]
                j = yoff[s] - yoff[olo]
                # fused bias + relu, per-partition bias scalar.  DVE is the
                # busiest engine mid-kernel, so tail slots split their two
                # chunk-relus across DVE and the (table-resident) ACT engine
                # to halve the post-last-matmul relu wall time.
                nc.vector.tensor_scalar(
                    yt[:, j:j + w], ps[:, 0:w],
                    bc[:, 2 * s:2 * s + 1], 0.0,
                    mybir.AluOpType.add, mybir.AluOpType.max,
                )
                if s >= S - 3:
                    nc.scalar.activation(
                        yt[:, j + w:j + 2 * w], ps[:, w:2 * w],
                        mybir.ActivationFunctionType.Relu,
                        bias=bc[:, 2 * s + 1:2 * s + 2],
                    )
                else:
                    nc.vector.tensor_scalar(
                        yt[:, j + w:j + 2 * w], ps[:, w:2 * w],
                        bc[:, 2 * s + 1:2 * s + 2], 0.0,
                        mybir.AluOpType.add, mybir.AluOpType.max,
                    )
                if s == ohi - 1:
                    gw = sum(2 * widths[t_] for t_ in range(olo, ohi))
                    # early y groups ride the otherwise-idle SWDGE queue;
                    # the next-to-last pair rides sync behind the inputs;
                    # the final (smallest) group rides scalar so its issue
                    # starts the moment its relus land
                    if og == len(ogroups) - 1:
                        oeng = nc.scalar
                    elif og >= len(ogroups) - 3:
                        oeng = nc.sync
                    else:
                        oeng = nc.gpsimd
                    oeng.dma_start(
                        y_d[:, yoff[olo]:yoff[olo] + gw], yt[:, 0:gw]
                    )
    nc.compile()
    return nc


def kernel(x: np.ndarray, idxs: np.ndarray, w: np.ndarray, b: np.ndarray) -> np.ndarray:
    global LAST_EXEC_TIME_NS, LAST_TRACE
    x = np.ascontiguousarray(x, dtype=np.float32)
    w = np.ascontiguousarray(w, dtype=np.float32)
    b = np.ascontiguousarray(b, dtype=np.float32)
    idxs_np = np.asarray(idxs).astype(np.int64)

    B = x.shape[0]
    order, per_core, widths = _build_schedule(idxs_np)
    S = len(widths)

    # per-expert weights in PE layout: [o, i] chunks packed [128, 512]
    # cols: (o0,i0)(o1,i0)(o0,i1)(o1,i1)
    wT = w.transpose(0, 2, 1).astype(MM_NP)  # (e, o, i)
    wprep = np.concatenate(
        [
            wT[:, 0:P, 0:P], wT[:, P:F, 0:P],
            wT[:, 0:P, P:F], wT[:, P:F, P:F],
        ],
        axis=2,
    )  # (e, 128, 512)

    xT = x.T.astype(MM_NP)  # (256, B)

    igroups = _in_groups(S)
    WSLOT = 4 * P

    nc = _build_program(S, widths)
    trace = bool(os.environ.get("KBENCH_TRACE"))

    in_maps = []
    for c in range(N_CORES):
        parts = []
        for lo, hi in igroups:
            for s in range(lo, hi):
                e, g0, cnt = per_core[c][s]
                ws = widths[s]
                parts.append(wprep[e])
                xs = np.zeros((2 * P, ws), dtype=MM_NP)
                if cnt:
                    xs[:, :cnt] = xT[:, order[g0:g0 + cnt]]
                parts.append(xs.reshape(2, P, ws).transpose(1, 0, 2).reshape(P, 2 * ws))
        xw = np.ascontiguousarray(np.concatenate(parts, axis=1))
        bcd = np.zeros((P, 2 * S), dtype=np.float32)
        for s in range(S):
            e = per_core[c][s][0]
            bcd[:, 2 * s] = b[e, 0:P]
            bcd[:, 2 * s + 1] = b[e, P:F]
        in_maps.append({"xw": xw, "bconst": bcd})

    res = run_bass_kernel_spmd(
        nc, in_maps, core_ids=list(range(N_CORES)), trace=trace
    )
    LAST_EXEC_TIME_NS = res.exec_time_ns
    LAST_TRACE = res.instructions_and_trace

    y = np.empty((B, F), dtype=np.float32)
    yoff = np.cumsum([0] + [2 * w_ for w_ in widths])
    for c in range(N_CORES):
        yT = res.results[c]["y"].astype(np.float32)  # (128, YTOT)
        for s in range(S):
            e, g0, cnt = per_core[c][s]
            if not cnt:
                continue
            sl = order[g0:g0 + cnt]
            o = yoff[s]
            ws = widths[s]
            y[sl, 0:P] = yT[:, o:o + cnt].T
            y[sl, P:F] = yT[:, o + ws:o + ws + cnt].T
    return y


# revision 17
# speedup vs baseline: 1.0926x; 1.0926x over previous
"""Trainium2 Bass kernel for batched per-sample expert matmul (MoE routing).

Computes y[n, i] = relu(b[idxs[n], i] + sum_o w[idxs[n], i, o] * x[n, o])
for x (8192, 256), idxs (8192,), w (64, 256, 256), b (64, 256).

Strategy (v2: expert-sharded, weight-stationary)
------------------------------------------------
Host side (numpy, cheap):
  * Group samples by expert; cut per-expert slots of <= 256 samples
    (one PSUM bank each); LPT-deal slots to the 8 cores balancing
    sample count.  Each expert's weight block is loaded ONCE per core
    (~1 MB/core vs 1.5 MB for slot-replicated loads), x/y ~0.5 MB each.
  * All matmul operands fp16 (absmax-rel err ~4e-4, gate is 2e-2).
  * Static SPMD program: slot widths W_s = max over cores of the
    rank-s slot's sample count (slots sorted desc), zero-padded.

Device side (one static Tile program on all 8 cores):
  Per slot s (expert e, W samples):  psum tile [128, 2W] (one bank)
      ps[i, n]      = sum_o wT[o, i]   * xT[o, n]      (4 matmuls:
        (o0,i0) start, (o1,i0), (o0,i1), (o1,i1) stop; weights are
        the stationary operand so PE cost scales with W, not 256)
      y[i', n]      = max(ps + b_col, 0)               (fused bias+relu,
        per-partition scalar; alternating DVE / ACT engines)
  Orientation note: output is yT (features on partitions); host
  transposes back.

  DMA plan: first slot group rides sync HWDGE (low first-byte latency);
  the bulk stream goes as few large packed x+w group DMAs on the gpsimd
  SWDGE queue (FIFO, one completion sem per group; issue cost ~0.65us
  each so groups hold >= 2 slots); bias consts ride scalar HWDGE.
  y outputs ride scalar HWDGE as their relus complete; the final small
  y group rides sync for the shortest last-receipt.

Host side: scatter yT columns back to sample order.
"""

import os

import numpy as np

import concourse.bacc as bacc
import concourse.bass as bass
import concourse.mybir as mybir
import concourse.tile as tile
from concourse.bass_utils import run_bass_kernel_spmd

N_CORES = 8
P = 128          # SBUF/PSUM partitions
F = 256          # feature dim (in == out == 256)
NEXP = 64
WMAX = 256       # samples per slot cap: psum tile [128, 2*W] fp32 <= one bank
MM_NP = np.float16
MM_DT = mybir.dt.float16
OUT_DT = mybir.dt.float16
F32 = mybir.dt.float32

# Set by the last kernel() call when KBENCH_TRACE=1 (used by test.py only).
LAST_EXEC_TIME_NS = None
LAST_TRACE = None


def _build_schedule(idxs: np.ndarray):
    """Sort samples by expert, cut <=WMAX-sample single-expert slots,
    LPT-deal slots to cores balancing total samples; sort each core's
    slots by count desc and compute static per-rank widths."""
    B = idxs.shape[0]
    order = np.argsort(idxs, kind="stable")
    sidx = idxs[order]
    slots = []  # (expert, global_start, count)
    i = 0
    while i < B:
        j = i
        while j < B and sidx[j] == sidx[i]:
            j += 1
        k = i
        while k < j:
            cnt = min(WMAX, j - k)
            slots.append((int(sidx[i]), k, cnt))
            k += cnt
        i = j
    # serpentine rank dealing: sort slots desc, rank r takes slots
    # [8r, 8r+8) (adjacent sizes -> minimal per-rank padding), serpentine
    # direction alternation balances per-core totals
    slots.sort(key=lambda t: -t[2])
    S = (len(slots) + N_CORES - 1) // N_CORES
    while len(slots) < S * N_CORES:
        slots.append((0, 0, 0))
    per_core = [[] for _ in range(N_CORES)]
    for r in range(S):
        rank = slots[r * N_CORES:(r + 1) * N_CORES]
        if r % 2 == 1:
            rank = rank[::-1]
        for c in range(N_CORES):
            per_core[c].append(rank[c])
    # static width per rank: max count across cores, multiple of 8, >= 8
    widths = []
    for s in range(S):
        w = max(pc[s][2] for pc in per_core)
        widths.append(max(8, (w + 7) // 8 * 8))
    return order, per_core, widths


def _in_groups(S):
    """Input stream slot groups: [1 (sync), 2s on gpsimd, single tail]."""
    out = [(0, 1)]
    lo = 1
    while lo < S - 1:
        hi = min(S - 1, lo + 2)
        out.append((lo, hi))
        lo = hi
    if S > 1:
        out.append((S - 1, S))
    return out


def _out_groups(S):
    """Output slot groups: pairs, except a single-slot tail group."""
    out = []
    lo = 0
    while lo < S - 1:
        hi = min(S - 1, lo + 2)
        out.append((lo, hi))
        lo = hi
    out.append((S - 1, S))
    return out


def _build_program(S, widths):
    nc = bacc.Bacc(
        "TRN2", target_bir_lowering=False, debug=False, num_devices=N_CORES
    )
    WSLOT = 4 * P  # w cols per slot (4 chunks of [128,128])
    cols = [WSLOT + 2 * w for w in widths]  # per-slot packed w+x cols
    igroups = _in_groups(S)
    ogroups = _out_groups(S)
    goff = []
    off = 0
    for lo, hi in igroups:
        goff.append(off)
        off += sum(cols[lo:hi])
    XWTOT = off
    yoff = []
    off = 0
    for s in range(S):
        yoff.append(off)
        off += 2 * widths[s]
    YTOT = off

    xw_d = nc.dram_tensor("xw", [P, XWTOT], MM_DT, kind="ExternalInput").ap()
    bc_d = nc.dram_tensor("bconst", [P, 2 * S], F32, kind="ExternalInput").ap()
    y_d = nc.dram_tensor("y", [P, YTOT], OUT_DT, kind="ExternalOutput").ap()

    with tile.TileContext(nc) as tc:
        with (
            tc.tile_pool(name="const", bufs=1) as const,
            tc.tile_pool(name="w", bufs=1) as wpool,
            tc.tile_pool(name="yout", bufs=1) as ypool,
            tc.tile_pool(name="ps", bufs=min(8, S), space="PSUM") as pspool,
        ):
            # bias columns on the scalar HWDGE ring (tiny, first in queue)
            bc = const.tile([P, 2 * S], F32, tag="bconst")
            with tc.high_priority():
                nc.scalar.dma_start(bc[:], bc_d[:])

            # all input groups ride the sync HWDGE ring: strict FIFO
            # delivery at line rate; only the LAST group's completion
            # receipt gates the tail
            gts = {}
            for g, (lo, hi) in enumerate(igroups):
                gw = sum(cols[lo:hi])
                t = wpool.tile([P, gw], MM_DT, tag=f"xw{g}", name=f"xw{g}")
                gts[g] = t
                if g == len(igroups) - 1 and hi - lo == 1:
                    # split last group into w-part and x-part for an earlier
                    # completion sem on the critical tail
                    wpart = 4 * P
                    nc.sync.dma_start(
                        t[:, 0:wpart], xw_d[:, goff[g]:goff[g] + wpart]
                    )
                    nc.sync.dma_start(
                        t[:, wpart:gw],
                        xw_d[:, goff[g] + wpart:goff[g] + gw],
                    )
                else:
                    nc.sync.dma_start(t[:], xw_d[:, goff[g]:goff[g] + gw])

            seg2g = {}
            for g, (lo, hi) in enumerate(igroups):
                for s in range(lo, hi):
                    seg2g[s] = g

            def wchunk(s, c):
                g = seg2g[s]
                lo, hi = igroups[g]
                base = sum(cols[lo:s]) + c * P
                return gts[g][:, base:base + P]

            def xchunk(s, c):
                g = seg2g[s]
                lo, hi = igroups[g]
                w = widths[s]
                base = sum(cols[lo:s]) + WSLOT + c * w
                return gts[g][:, base:base + w]

            seg2o = {}
            for g, (lo, hi) in enumerate(ogroups):
                for s in range(lo, hi):
                    seg2o[s] = g
            yts = {}

            for s in range(S):
                w = widths[s]
                ps = pspool.tile([P, 512], F32, name="ps")
                # 4 matmuls: (o0,i0) start, (o1,i0), (o0,i1), (o1,i1) stop
                nc.tensor.matmul(
                    ps[:, 0:w], wchunk(s, 0), xchunk(s, 0),
                    start=True, stop=False,
                )
                nc.tensor.matmul(
                    ps[:, 0:w], wchunk(s, 1), xchunk(s, 1),
                    start=False, stop=False,
                )
                nc.tensor.matmul(
                    ps[:, w:2 * w], wchunk(s, 2), xchunk(s, 0),
                    start=False, stop=False,
                )
                nc.tensor.matmul(
                    ps[:, w:2 * w], wchunk(s, 3), xchunk(s, 1),
                    start=False, stop=True,
                )

                og = seg2o[s]
                olo, ohi = ogroups[og]
                if og not in yts:
                    gw = sum(2 * widths[t_] for t_ in range(olo, ohi))
                    yts[og] = ypool.tile(
                        [P, gw], OUT_DT, tag=f"y{og}", name=f"y{og}"
                    )
                yt = yts[og]
                j = yoff[s] - yoff[olo]
                # fused bias + relu on DVE (per-partition bias scalar)
                nc.vector.tensor_scalar(
                    yt[:, j:j + w], ps[:, 0:w],
                    bc[:, 2 * s:2 * s + 1], 0.0,
                    mybir.AluOpType.add, mybir.AluOpType.max,
                )
                nc.vector.tensor_scalar(
                    yt[:, j + w:j + 2 * w], ps[:, w:2 * w],
                    bc[:, 2 * s + 1:2 * s + 2], 0.0,
                    mybir.AluOpType.add, mybir.AluOpType.max,
                )
                if s == ohi - 1:
                    gw = sum(2 * widths[t_] for t_ in range(olo, ohi))
                    # early y groups ride the scalar HWDGE ring (its queue is
                    # otherwise idle); the final (smallest) group rides sync
                    # so its issue starts the moment its relus land
                    oeng = nc.sync if og == len(ogroups) - 1 else nc.scalar
                    oeng.dma_start(
                        y_d[:, yoff[olo]:yoff[olo] + gw], yt[:, 0:gw]
                    )
    nc.compile()
    return nc


def kernel(x: np.ndarray, idxs: np.ndarray, w: np.ndarray, b: np.ndarray) -> np.ndarray:
    global LAST_EXEC_TIME_NS, LAST_TRACE
    x = np.ascontiguousarray(x, dtype=np.float32)
    w = np.ascontiguousarray(w, dtype=np.float32)
    b = np.ascontiguousarray(b, dtype=np.float32)
    idxs_np = np.asarray(idxs).astype(np.int64)

    B = x.shape[0]
    order, per_core, widths = _build_schedule(idxs_np)
    S = len(widths)

    # per-expert weights in PE layout: [o, i] chunks packed [128, 512]
    # cols: (o0,i0)(o1,i0)(o0,i1)(o1,i1)
    wT = w.transpose(0, 2, 1).astype(MM_NP)  # (e, o, i)
    wprep = np.concatenate(
        [
            wT[:, 0:P, 0:P], wT[:, P:F, 0:P],
            wT[:, 0:P, P:F], wT[:, P:F, P:F],
        ],
        axis=2,
    )  # (e, 128, 512)

    xT = x.T.astype(MM_NP)  # (256, B)

    igroups = _in_groups(S)
    WSLOT = 4 * P

    nc = _build_program(S, widths)
    trace = bool(os.environ.get("KBENCH_TRACE"))

    in_maps = []
    for c in range(N_CORES):
        parts = []
        for lo, hi in igroups:
            for s in range(lo, hi):
                e, g0, cnt = per_core[c][s]
                ws = widths[s]
                parts.append(wprep[e])
                xs = np.zeros((2 * P, ws), dtype=MM_NP)
                if cnt:
                    xs[:, :cnt] = xT[:, order[g0:g0 + cnt]]
                parts.append(xs.reshape(2, P, ws).transpose(1, 0, 2).reshape(P, 2 * ws))
        xw = np.ascontiguousarray(np.concatenate(parts, axis=1))
        bcd = np.zeros((P, 2 * S), dtype=np.float32)
        for s in range(S):
            e = per_core[c][s][0]
            bcd[:, 2 * s] = b[e, 0:P]
            bcd[:, 2 * s + 1] = b[e, P:F]
        in_maps.append({"xw": xw, "bconst": bcd})

    res = run_bass_kernel_spmd(
        nc, in_maps, core_ids=list(range(N_CORES)), trace=trace
    )
    LAST_EXEC_TIME_NS = res.exec_time_ns
    LAST_TRACE = res.instructions_and_trace

    y = np.empty((B, F), dtype=np.float32)
    yoff = np.cumsum([0] + [2 * w_ for w_ in widths])
    for c in range(N_CORES):
        yT = res.results[c]["y"].astype(np.float32)  # (128, YTOT)
        for s in range(S):
            e, g0, cnt = per_core[c][s]
            if not cnt:
                continue
            sl = order[g0:g0 + cnt]
            o = yoff[s]
            ws = widths[s]
            y[sl, 0:P] = yT[:, o:o + cnt].T
            y[sl, P:F] = yT[:, o + ws:o + ws + cnt].T
    return y
